# revision 1
# baseline (speedup 1.0000x reference)
"""BEiT window attention (B=8, N=1024, C=768, 12 heads) on 8 TRN2 NeuronCores.

Sharding: pure data-parallel over batch — one batch element per core, no
collectives. Per-core dataflow (bf16 matmuls, f32 PSUM accumulation):

  qT/kT computed feature-major (transposed) so the attention scores are
  produced directly as S^T (keys on partitions, queries free) and softmax
  needs no on-device transposes.  P = exp(S^T) * E^T with E = exp(rel-pos
  bias) precomputed host-side (it is a gather of a 3972x12 table).  Softmax
  denominators come from an all-ones block appended to the stationary V
  operand (rows 64..127 of the attn@v PSUM tile hold the replicated row
  sums).  Normalization = reciprocal + partition-broadcast DMA + one
  tensor_tensor multiply.  The projection consumes O^T directly and the
  host transposes the final (768, 1024) result back.
"""

import sys
import types

import numpy as np
import ml_dtypes

BF16NP = ml_dtypes.bfloat16

P = 128        # partitions
NTOK = 1024    # tokens per batch element
C = 768        # embed dim
NH = 12        # heads
HD = 64        # head dim
NPAIR = 6      # head pairs
NQT = 2        # query tiles of 512
QW = 512       # query tile width
KC = 8         # key chunks of 128
NCORES = 8


def _install_axon_hooks():
    """Register the NTFF profile hook module missing from this image's antenv."""
    if "antenv.axon_hooks" in sys.modules:
        return
    try:
        import antenv  # noqa: F401
        from trn_agent_boot.trn_boot import _ntff_profile_via_ctypes

        mod = types.ModuleType("antenv.axon_hooks")
        mod._hook = _ntff_profile_via_ctypes("/opt/axon/libaxon_pjrt.so")
        mod.get_axon_ntff_profile_hook = lambda: mod._hook
        mod.set_axon_ntff_profile_hook = lambda h: setattr(mod, "_hook", h)
        sys.modules["antenv.axon_hooks"] = mod
    except Exception:
        pass


_BUILD_CACHE = {}


def _build():
    if "nc" in _BUILD_CACHE:
        return _BUILD_CACHE["nc"]

    from contextlib import ExitStack

    import concourse.bass as bass
    import concourse.bacc as bacc
    import concourse.mybir as mybir
    import concourse.tile as tile

    BF = mybir.dt.bfloat16
    F32 = mybir.dt.float32
    AF = mybir.ActivationFunctionType

    nc = bacc.Bacc("TRN2", target_bir_lowering=False, debug=False)

    xT_d = nc.dram_tensor("xT", [C, NTOK], BF, kind="ExternalInput").ap()
    wqkvT_d = nc.dram_tensor("wqkvT", [C, 3 * C], BF, kind="ExternalInput").ap()
    qkb_d = nc.dram_tensor("qkb", [P, 12], F32, kind="ExternalInput").ap()
    vb_d = nc.dram_tensor("vb", [1, C], BF, kind="ExternalInput").ap()
    # (pair, qtile, kchunk, key-part, head-in-pair, q)
    ET_d = nc.dram_tensor(
        "ET", [NPAIR, NQT, KC, P, 2, QW], BF, kind="ExternalInput"
    ).ap()
    pwT_d = nc.dram_tensor("pwT", [C, C], BF, kind="ExternalInput").ap()
    pbT_d = nc.dram_tensor("pbT", [P, 6], F32, kind="ExternalInput").ap()
    out_d = nc.dram_tensor("out", [C, NTOK], F32, kind="ExternalOutput").ap()

    with ExitStack() as ctx:
        tc = ctx.enter_context(tile.TileContext(nc))
        const = ctx.enter_context(tc.tile_pool(name="const", bufs=1))
        spool = ctx.enter_context(tc.tile_pool(name="spool", bufs=2, space="PSUM"))
        opool = ctx.enter_context(tc.tile_pool(name="opool", bufs=3, space="PSUM"))
        epool = ctx.enter_context(tc.tile_pool(name="epool", bufs=3))
        prawp = ctx.enter_context(tc.tile_pool(name="praw", bufs=3))
        pfinp = ctx.enter_context(tc.tile_pool(name="pfin", bufs=3))
        smallp = ctx.enter_context(tc.tile_pool(name="small", bufs=4))
        dramp = ctx.enter_context(tc.tile_pool(name="dram", bufs=4, space="DRAM"))
        youtp = ctx.enter_context(tc.tile_pool(name="yout", bufs=2))

        # ---- persistent SBUF tensors ----
        xT_sb = const.tile([P, 6, NTOK], BF)          # x^T, feature-major
        w_sb = const.tile([P, 6, 3 * C], BF)          # qkv_w^T (q cols pre-scaled)
        qk_sb = const.tile([P, 12, NTOK], BF)         # q^T (chunks 0-5), k^T (6-11)
        v_sb = const.tile([P, KC, NH * (HD + 1)], BF)  # 12x[v_h|1] blocks per kchunk
        op_sb = const.tile([P, NPAIR, NTOK], BF)      # normalized O^T, pair-stacked
        pw_sb = const.tile([P, 6, C], BF)             # proj_w^T
        qkb_sb = const.tile([P, 12], F32)
        pb_sb = const.tile([P, 6], F32)
        vb_sb = const.tile([P, C], BF)

        for c in range(6):
            nc.sync.dma_start(out=xT_sb[:, c, :], in_=xT_d[c * P:(c + 1) * P, :])
            nc.sync.dma_start(out=w_sb[:, c, :], in_=wqkvT_d[c * P:(c + 1) * P, :])
            nc.sync.dma_start(out=pw_sb[:, c, :], in_=pwT_d[c * P:(c + 1) * P, :])
        nc.sync.dma_start(out=qkb_sb[:], in_=qkb_d[:])
        nc.sync.dma_start(out=pb_sb[:], in_=pbT_d[:])
        nc.sync.dma_start(out=vb_sb[:], in_=vb_d.broadcast_to((P, C)))
        nc.vector.memset(v_sb[:], 1.0)  # ones columns; v blocks overwritten below

        # ---- phase 1: qkv projections ----
        def emit_qk(j):
            ps = spool.tile([P, NTOK], F32, tag="s")
            for c in range(6):
                for half in range(2):
                    sl = slice(half * 512, (half + 1) * 512)
                    nc.tensor.matmul(
                        ps[:, sl],
                        lhsT=w_sb[:, c, j * P:(j + 1) * P],
                        rhs=xT_sb[:, c, sl],
                        start=(c == 0),
                        stop=(c == 5),
                    )
            # copy+bias on ScalarE (idle in phase 1); DVE is busy later
            nc.scalar.activation(
                out=qk_sb[:, j, :], in_=ps[:], func=AF.Identity,
                bias=qkb_sb[:, j:j + 1],
            )

        def emit_v(t):
            ps = spool.tile([P, NTOK], F32, tag="s")
            for c in range(6):
                for off, width in ((0, 512), (512, 256)):
                    nc.tensor.matmul(
                        ps[:, off:off + width],
                        lhsT=xT_sb[:, c, t * P:(t + 1) * P],
                        rhs=w_sb[:, c, 2 * C + off:2 * C + off + width],
                        start=(c == 0),
                        stop=(c == 5),
                    )
            nc.vector.tensor_add(
                v_sb[:, t, :].rearrange("p (h w) -> p h w", w=HD + 1)[:, :, 0:HD],
                ps[:, 0:C].rearrange("p (h w) -> p h w", w=HD),
                vb_sb[:].rearrange("p (h w) -> p h w", w=HD),
            )

        emit_qk(0)
        emit_qk(6)
        for t in range(KC):
            emit_v(t)
        for p in range(1, 6):
            emit_qk(p)
            emit_qk(6 + p)

        # ---- phase 2: attention ----
        for p in range(NPAIR):
            for qt in range(NQT):
                oA = opool.tile([P, QW], F32, tag="o")
                oB = opool.tile([P, QW], F32, tag="o")
                for kc in range(KC):
                    e_sb = epool.tile([P, NTOK], BF)
                    nc.sync.dma_start(
                        out=e_sb[:],
                        in_=ET_d[p:p + 1, qt:qt + 1, kc:kc + 1, :, :, :].rearrange(
                            "a b c p h q -> (a p) (b c h q)"
                        ),
                    )
                    s_ps = spool.tile([P, NTOK], F32, tag="s")
                    for h in range(2):
                        hh = HD * h
                        nc.tensor.matmul(
                            s_ps[:, h * QW:(h + 1) * QW],
                            lhsT=qk_sb[hh:hh + HD, 6 + p, kc * P:(kc + 1) * P],
                            rhs=qk_sb[hh:hh + HD, p, qt * QW:(qt + 1) * QW],
                            start=True,
                            stop=True,
                        )
                    praw = prawp.tile([P, NTOK], BF)
                    nc.scalar.activation(out=praw[:], in_=s_ps[:], func=AF.Exp)
                    ptile = pfinp.tile([P, NTOK], BF)
                    nc.vector.tensor_mul(ptile[:], praw[:], e_sb[:])
                    for h in range(2):
                        head = 2 * p + h
                        o_ps = oA if h == 0 else oB
                        nc.tensor.matmul(
                            o_ps[0:HD + 1, :],
                            lhsT=v_sb[:, kc, (HD + 1) * head:(HD + 1) * (head + 1)],
                            rhs=ptile[:, h * QW:(h + 1) * QW],
                            start=(kc == 0),
                            stop=(kc == KC - 1),
                        )
                # normalization, decoupled from the PSUM accumulators:
                # one fast copy frees the o slot; recip/broadcast/mul run on
                # the SBUF copy off the PE critical path.
                for h in range(2):
                    o_ps = oA if h == 0 else oB
                    ocp = smallp.tile([P, QW], F32, tag="ocp")
                    nc.vector.tensor_copy(ocp[0:HD + 1, :], o_ps[0:HD + 1, :])
                    # spread the 512 sums over 8 partitions so the iterative
                    # divide runs 8 lanes wide (658ns vs 3.3us)
                    rsp = smallp.tile([8, HD], F32, tag="rsp")
                    nc.sync.dma_start(out=rsp[:], in_=ocp[HD:HD + 1, :])
                    rinv = smallp.tile([8, HD], F32, tag="rinv")
                    nc.vector.reciprocal(rinv[:], rsp[:])
                    sinv = smallp.tile([P, QW], F32, tag="sinv")
                    dscratch = dramp.tile([1, QW], F32, tag="ds")
                    nc.sync.dma_start(out=dscratch[:], in_=rinv[:])
                    nc.sync.dma_start(
                        out=sinv[0:HD, :],
                        in_=dscratch[:].broadcast_to((HD, QW)),
                    )
                    if h == 0:
                        nc.vector.tensor_mul(
                            op_sb[0:HD, p, qt * QW:(qt + 1) * QW],
                            ocp[0:HD, :],
                            sinv[0:HD, :],
                        )
                    else:
                        stage = smallp.tile([P, QW], BF, tag="stage")
                        nc.vector.tensor_mul(
                            stage[0:HD, :], ocp[0:HD, :], sinv[0:HD, :]
                        )
                        nc.sync.dma_start(
                            out=op_sb[HD:P, p, qt * QW:(qt + 1) * QW],
                            in_=stage[0:HD, :],
                        )

        # ---- phase 3: output projection ----
        for ec in range(6):
            ps = spool.tile([P, NTOK], F32, tag="s")
            for p in range(NPAIR):
                for nt in range(2):
                    sl = slice(nt * 512, (nt + 1) * 512)
                    nc.tensor.matmul(
                        ps[:, sl],
                        lhsT=pw_sb[:, p, ec * P:(ec + 1) * P],
                        rhs=op_sb[:, p, sl],
                        start=(p == 0),
                        stop=(p == NPAIR - 1),
                    )
            y_sb = youtp.tile([P, NTOK], F32)
            nc.vector.tensor_scalar_add(y_sb[:], ps[:], pb_sb[:, ec:ec + 1])
            nc.sync.dma_start(out=out_d[ec * P:(ec + 1) * P, :], in_=y_sb[:])

    nc.compile()
    _BUILD_CACHE["nc"] = nc
    return nc


def _prep_inputs(x, qkv_w, q_bias, v_bias, rel_bias_table, proj_w, proj_b,
                 rel_pos_idx):
    x = np.asarray(x, np.float32)
    qkv_w = np.asarray(qkv_w, np.float32)
    q_bias = np.asarray(q_bias, np.float32)
    v_bias = np.asarray(v_bias, np.float32)
    rel_bias_table = np.asarray(rel_bias_table, np.float32)
    proj_w = np.asarray(proj_w, np.float32)
    proj_b = np.asarray(proj_b, np.float32)
    rel_pos_idx = np.asarray(rel_pos_idx, np.int64)

    scale = HD ** -0.5
    wq = qkv_w[:C] * scale
    wqkvT = np.ascontiguousarray(
        np.concatenate([wq, qkv_w[C:]], axis=0).T
    ).astype(BF16NP)

    qk_bias = np.concatenate([q_bias * scale, np.zeros(C, np.float32)])
    qkb = np.ascontiguousarray(qk_bias.reshape(12, P).T)

    vb = v_bias.astype(BF16NP).reshape(1, C)

    # E^T[h, m, n] = exp(bias[h, n, m]); bias[h, n, m] = table[idx[n, m], h]
    A = np.exp(rel_bias_table)[rel_pos_idx]            # (n, m, h)
    ETpre = A.transpose(2, 1, 0)                       # (h, m, n)
    ET = np.ascontiguousarray(
        ETpre.reshape(NPAIR, 2, KC, P, NQT, QW).transpose(0, 4, 2, 3, 1, 5)
    ).astype(BF16NP)

    pwT = np.ascontiguousarray(proj_w.T).astype(BF16NP)
    pbT = np.ascontiguousarray(proj_b.reshape(6, P).T)

    shared = {
        "wqkvT": wqkvT, "qkb": qkb, "vb": vb, "ET": ET,
        "pwT": pwT, "pbT": pbT,
    }
    in_maps = []
    xb16 = x.reshape(NCORES, NTOK, C).astype(BF16NP)
    for b in range(NCORES):
        m = dict(shared)
        m["xT"] = np.ascontiguousarray(xb16[b].T)
        in_maps.append(m)
    return in_maps


def _run(inputs, trace=False):
    import time as _time

    _install_axon_hooks()
    from concourse.bass_utils import run_bass_kernel_spmd

    t0 = _time.time()
    nc = _build()
    print(f"[kernel] build+compile: {_time.time() - t0:.1f}s", flush=True)
    t0 = _time.time()
    in_maps = _prep_inputs(**inputs)
    print(f"[kernel] host prep: {_time.time() - t0:.1f}s", flush=True)
    t0 = _time.time()
    res = run_bass_kernel_spmd(
        nc, in_maps, core_ids=list(range(NCORES)), trace=trace
    )
    print(f"[kernel] hw run: {_time.time() - t0:.1f}s", flush=True)
    outs = [np.asarray(res.results[b]["out"]) for b in range(NCORES)]
    y = np.stack([o.T.reshape(32, 32, C) for o in outs]).astype(np.float32)
    return y, res


def kernel(**inputs) -> np.ndarray:
    y, _ = _run(inputs, trace=False)
    return y



# revision 6
# speedup vs baseline: 1.1960x; 1.1960x over previous
"""BEiT window attention (B=8, N=1024, C=768, 12 heads) on 8 TRN2 NeuronCores.

Sharding: pure data-parallel over batch - one batch element per core, no
collectives.  v2: single software-pipelined emission.

Per-core dataflow (bf16 matmuls, f32 PSUM accumulation): qT/kT computed
feature-major so attention scores are produced directly as S^T (keys on
partitions) and softmax needs no on-device transposes.  P = exp(S^T) * E^T
with E = exp(rel-pos bias) precomputed host-side.  Softmax denominators come
from an all-ones column appended to each V block (row 64 of the attn@v PSUM
accumulators).

v2 structure (vs v1's 3 serial phases):
 - Only qk chunks 0/6 and V chunks 0-5 are computed up front; V6/V7 and the
   ten remaining qkv-projection chunks are interleaved into the attention
   units as PE filler so the tensor engine never idles (and never drops out
   of its high p-state) while the Scalar engine paces the exp chain.
 - Scalar engine runs ONLY the 96 exp ACTs (the pacer at ~1.33us each).
 - exp(S)*E multiplies split between DVE and GpSimd (all-SBUF, so Pool can
   take them); PSUM evacuation (qk copies, o copies) stays on DVE.
 - attn@v is skewed 3 beats behind the scores matmuls so its semaphores are
   always satisfied when the PE reaches it.
 - PSUM budget exactly 8 banks: scores 2x2, qkv-filler 1x2, o-accum 2x1.
"""

import sys
import types

import numpy as np
import ml_dtypes

BF16NP = ml_dtypes.bfloat16

P = 128        # partitions
NTOK = 1024    # tokens per batch element
C = 768        # embed dim
NH = 12        # heads
HD = 64        # head dim
NPAIR = 6      # head pairs
NQT = 2        # query tiles of 512
QW = 512       # query tile width
KC = 8         # key chunks of 128
NCORES = 8
SKEW = 3       # beats between scores and attn@v consumption
NBEAT = NPAIR * NQT * KC  # 96


def _install_axon_hooks():
    """Register the NTFF profile hook module missing from this image's antenv."""
    if "antenv.axon_hooks" in sys.modules:
        return
    try:
        import antenv  # noqa: F401
        from trn_agent_boot.trn_boot import _ntff_profile_via_ctypes

        mod = types.ModuleType("antenv.axon_hooks")
        mod._hook = _ntff_profile_via_ctypes("/opt/axon/libaxon_pjrt.so")
        mod.get_axon_ntff_profile_hook = lambda: mod._hook
        mod.set_axon_ntff_profile_hook = lambda h: setattr(mod, "_hook", h)
        sys.modules["antenv.axon_hooks"] = mod
    except Exception:
        pass


_BUILD_CACHE = {}


def _build():
    if "nc" in _BUILD_CACHE:
        return _BUILD_CACHE["nc"]

    from contextlib import ExitStack

    import concourse.bass as bass
    import concourse.bacc as bacc
    import concourse.mybir as mybir
    import concourse.tile as tile

    BF = mybir.dt.bfloat16
    F32 = mybir.dt.float32
    AF = mybir.ActivationFunctionType

    nc = bacc.Bacc("TRN2", target_bir_lowering=False, debug=False)

    xT_d = nc.dram_tensor("xT", [C, NTOK], BF, kind="ExternalInput").ap()
    wqkvT_d = nc.dram_tensor("wqkvT", [C, 3 * C], BF, kind="ExternalInput").ap()
    qkb_d = nc.dram_tensor("qkb", [P, 12], F32, kind="ExternalInput").ap()
    vb_d = nc.dram_tensor("vb", [1, C], BF, kind="ExternalInput").ap()
    # (pair, qtile, kchunk, key-part, head-in-pair, q)
    ET_d = nc.dram_tensor(
        "ET", [NPAIR, NQT, KC, P, 2, QW], BF, kind="ExternalInput"
    ).ap()
    pwT_d = nc.dram_tensor("pwT", [C, C], BF, kind="ExternalInput").ap()
    pbT_d = nc.dram_tensor("pbT", [P, 6], F32, kind="ExternalInput").ap()
    out_d = nc.dram_tensor("out", [C, NTOK], F32, kind="ExternalOutput").ap()

    units = [(p, qt) for p in range(NPAIR) for qt in range(NQT)]

    with ExitStack() as ctx:
        tc = ctx.enter_context(tile.TileContext(nc))
        const = ctx.enter_context(tc.tile_pool(name="const", bufs=1))
        spool = ctx.enter_context(tc.tile_pool(name="spool", bufs=2, space="PSUM"))
        qkpool = ctx.enter_context(tc.tile_pool(name="qkps", bufs=1, space="PSUM"))
        opool = ctx.enter_context(tc.tile_pool(name="opool", bufs=2, space="PSUM"))
        epool = ctx.enter_context(tc.tile_pool(name="epool", bufs=4))
        prawp = ctx.enter_context(tc.tile_pool(name="praw", bufs=4))
        pfinp = ctx.enter_context(tc.tile_pool(name="pfin", bufs=4))
        ocpp = ctx.enter_context(tc.tile_pool(name="ocp", bufs=2))
        smallp = ctx.enter_context(tc.tile_pool(name="small", bufs=4))
        sinvp = ctx.enter_context(tc.tile_pool(name="sinv", bufs=2))
        stgp = ctx.enter_context(tc.tile_pool(name="stg", bufs=2))
        dramp = ctx.enter_context(tc.tile_pool(name="dram", bufs=4, space="DRAM"))
        youtp = ctx.enter_context(tc.tile_pool(name="yout", bufs=2))

        # ---- persistent SBUF tensors ----
        xT_sb = const.tile([P, 6, NTOK], BF)          # x^T, feature-major
        w_sb = const.tile([P, 6, 3 * C], BF)          # qkv_w^T (q cols pre-scaled)
        qk_sb = const.tile([P, 12, NTOK], BF)         # q^T (chunks 0-5), k^T (6-11)
        v_sb = const.tile([P, KC, NH * (HD + 1)], BF)  # 12x[v_h|1] blocks per kchunk
        op_sb = const.tile([P, NPAIR, NTOK], BF)      # normalized O^T, pair-stacked
        pw_sb = const.tile([P, 6, C], BF)             # proj_w^T
        qkb_sb = const.tile([P, 12], F32)
        pb_sb = const.tile([P, 6], F32)
        vb_sb = const.tile([P, C], BF)

        # ---- input DMAs, priority order (single in-order SP queue) ----
        nc.sync.dma_start(
            out=xT_sb[:], in_=xT_d.rearrange("(c p) n -> p c n", p=P)
        )
        w_r = wqkvT_d.rearrange("(c p) m -> p c m", p=P)
        nc.sync.dma_start(out=w_sb[:, :, 0:C], in_=w_r[:, :, 0:C])
        nc.sync.dma_start(out=w_sb[:, :, C:2 * C], in_=w_r[:, :, C:2 * C])
        nc.sync.dma_start(out=qkb_sb[:], in_=qkb_d[:])
        nc.sync.dma_start(out=vb_sb[:], in_=vb_d.broadcast_to((P, C)))
        nc.sync.dma_start(out=w_sb[:, :, 2 * C:3 * C], in_=w_r[:, :, 2 * C:3 * C])

        # ones columns of the V blocks (softmax denominator trick)
        nc.gpsimd.memset(
            v_sb[:].rearrange("p k (h w) -> p k h w", w=HD + 1)[:, :, :, HD:HD + 1],
            1.0,
        )

        # ---- building blocks ----
        def qk_matmul_ops(j, pool, cell, tag):
            ops = []

            def get_tile():
                if "ps" not in cell:
                    cell["ps"] = pool.tile(
                        [P, NTOK], F32, tag=tag, name=tag
                    )
                return cell["ps"]

            for c in range(6):
                for half in range(2):
                    def op(c=c, half=half):
                        ps = get_tile()
                        sl = slice(half * QW, (half + 1) * QW)
                        nc.tensor.matmul(
                            ps[:, sl],
                            lhsT=w_sb[:, c, j * P:(j + 1) * P],
                            rhs=xT_sb[:, c, sl],
                            start=(c == 0),
                            stop=(c == 5),
                        )
                    ops.append(op)
            return ops

        def qk_copy_scalar(j, cell):
            # upfront only: Scalar is idle before the exp chain starts
            nc.scalar.activation(
                out=qk_sb[:, j, :], in_=cell["ps"][:], func=AF.Identity,
                bias=qkb_sb[:, j:j + 1],
            )

        def qk_copy_vector(j, cell):
            nc.vector.tensor_scalar_add(
                qk_sb[:, j, :], cell["ps"][:], qkb_sb[:, j:j + 1]
            )

        def v_matmul_ops(t, pool, cell, tag):
            ops = []

            def get_tile():
                if "ps" not in cell:
                    cell["ps"] = pool.tile(
                        [P, NTOK], F32, tag=tag, name=tag
                    )
                return cell["ps"]

            for c in range(6):
                for off, width in ((0, 512), (512, 256)):
                    def op(c=c, off=off, width=width):
                        ps = get_tile()
                        nc.tensor.matmul(
                            ps[:, off:off + width],
                            lhsT=xT_sb[:, c, t * P:(t + 1) * P],
                            rhs=w_sb[:, c, 2 * C + off:2 * C + off + width],
                            start=(c == 0),
                            stop=(c == 5),
                        )
                    ops.append(op)
            return ops

        def v_add(t, cell):
            nc.vector.tensor_add(
                v_sb[:, t, :].rearrange("p (h w) -> p h w", w=HD + 1)[:, :, 0:HD],
                cell["ps"][:, 0:C].rearrange("p (h w) -> p h w", w=HD),
                vb_sb[:].rearrange("p (h w) -> p h w", w=HD),
            )

        # ---- upfront: qk(0), qk(6), V(0..5) ----
        for j in (0, 6):
            cell = {}
            for op in qk_matmul_ops(j, spool, cell, "spool"):
                op()
            qk_copy_scalar(j, cell)
        for t in range(6):
            cell = {}
            for op in v_matmul_ops(t, spool, cell, "spool"):
                op()
            v_add(t, cell)

        # ---- fill plan: PE work interleaved into the attention beats ----
        def make_qk_fill(j):
            cell = {}
            ops = qk_matmul_ops(j, qkpool, cell, "fillps")
            ops.append(lambda: qk_copy_vector(j, cell))
            return ops

        def make_v_fill(t):
            cell = {}
            ops = v_matmul_ops(t, qkpool, cell, "fillps")
            ops.append(lambda: v_add(t, cell))
            return ops

        fill_plan = {
            0: [("v", 6), ("v", 7)],
            1: [("qk", 7), ("qk", 1)],
            2: [("qk", 8)], 3: [("qk", 2)],
            4: [("qk", 9)], 5: [("qk", 3)],
            6: [("qk", 10)], 7: [("qk", 4)],
            8: [("qk", 11)], 9: [("qk", 5)],
            10: [], 11: [],
        }
        fills_by_beat = {}  # global beat -> [callables]
        for u, chunks in fill_plan.items():
            ops = []
            for kind, idx in chunks:
                ops.extend(make_v_fill(idx) if kind == "v" else make_qk_fill(idx))
            if not ops:
                continue
            per_beat = -(-len(ops) // 8)
            for i, op in enumerate(ops):
                g = u * 8 + min(i // per_beat, 7)
                fills_by_beat.setdefault(g, []).append(op)

        # ---- flat attention loop: 96 beats + SKEW trailing ----
        e_tiles = {}
        ptiles = {}
        unit_state = {}
        deferred = {}  # global beat -> [callables]

        def issue_e(g):
            p, qt = units[g // 8]
            kc = g % 8
            e_sb = epool.tile([P, NTOK], BF, tag="e")
            nc.sync.dma_start(
                out=e_sb[:],
                in_=ET_d[p:p + 1, qt:qt + 1, kc:kc + 1, :, :, :].rearrange(
                    "a b c p h q -> (a p) (b c h q)"
                ),
            )
            e_tiles[g] = e_sb

        def norm_a(u):
            """Evacuate o accumulators + start the reciprocal/broadcast chain."""
            st = unit_state[u]
            ocp = ocpp.tile([P, NTOK], F32, tag="ocp")
            nc.vector.tensor_copy(ocp[0:HD + 1, 0:QW], st["oA"][0:HD + 1, :])
            nc.vector.tensor_copy(ocp[0:HD + 1, QW:NTOK], st["oB"][0:HD + 1, :])
            # spread the 1024 sums over 8 partitions so the iterative divide
            # runs 8 lanes wide
            rsp = smallp.tile([8, P], F32, tag="rsp")
            nc.gpsimd.dma_start(out=rsp[:], in_=ocp[HD:HD + 1, :])
            rinv = smallp.tile([8, P], F32, tag="rinv")
            nc.vector.reciprocal(rinv[:], rsp[:])
            dscr = dramp.tile([1, NTOK], F32, tag="ds")
            nc.gpsimd.dma_start(out=dscr[:], in_=rinv[:])
            sinv = sinvp.tile([HD, NTOK], F32, tag="sinv")
            nc.gpsimd.dma_start(
                out=sinv[:], in_=dscr[:].broadcast_to((HD, NTOK))
            )
            st["ocp"] = ocp
            st["sinv"] = sinv

        def norm_b(u):
            """Normalize into op_sb (deferred so the DVE never blocks on the
            reciprocal/broadcast latency chain)."""
            p, qt = units[u]
            st = unit_state[u]
            ocp, sinv = st["ocp"], st["sinv"]
            nc.vector.tensor_mul(
                op_sb[0:HD, p, qt * QW:(qt + 1) * QW],
                ocp[0:HD, 0:QW],
                sinv[:, 0:QW],
            )
            stage = stgp.tile([HD, QW], BF, tag="stage")
            nc.vector.tensor_mul(stage[:], ocp[0:HD, QW:NTOK], sinv[:, QW:NTOK])
            nc.gpsimd.dma_start(
                out=op_sb[HD:P, p, qt * QW:(qt + 1) * QW], in_=stage[:]
            )

        issue_e(0)
        issue_e(1)
        for g in range(NBEAT + SKEW):
            for op in deferred.pop(g, ()):
                op()
            if g < NBEAT:
                if g + 2 < NBEAT:
                    issue_e(g + 2)
                p, qt = units[g // 8]
                kc = g % 8
                # scores: S^T tile [128 keys, 2 heads x 512 queries]
                s_ps = spool.tile([P, NTOK], F32, tag="spool")
                for h in range(2):
                    hh = HD * h
                    nc.tensor.matmul(
                        s_ps[:, h * QW:(h + 1) * QW],
                        lhsT=qk_sb[hh:hh + HD, 6 + p, kc * P:(kc + 1) * P],
                        rhs=qk_sb[hh:hh + HD, p, qt * QW:(qt + 1) * QW],
                        start=True,
                        stop=True,
                    )
                praw = prawp.tile([P, NTOK], BF, tag="praw")
                nc.scalar.activation(out=praw[:], in_=s_ps[:], func=AF.Exp)
                ptile = pfinp.tile([P, NTOK], BF, tag="pfin")
                eng = nc.gpsimd if kc in (0, 6, 7) else nc.vector
                eng.tensor_mul(ptile[:], praw[:], e_tiles.pop(g)[:])
                ptiles[g] = ptile
                for op in fills_by_beat.pop(g, ()):
                    op()
            if g == 72:
                # proj weights, needed from ~t=150us; issue mid-stream
                nc.sync.dma_start(
                    out=pw_sb[:], in_=pwT_d.rearrange("(c p) m -> p c m", p=P)
                )
                nc.sync.dma_start(out=pb_sb[:], in_=pbT_d[:])
            b = g - SKEW
            if b >= 0:
                u = b // 8
                kcb = b % 8
                if kcb == 0:
                    oA = opool.tile([P, QW], F32, tag="o", name="oA")
                    oB = opool.tile([P, QW], F32, tag="o", name="oB")
                    unit_state[u] = {"oA": oA, "oB": oB}
                st = unit_state[u]
                pt = ptiles.pop(b)
                pu, _ = units[u]
                for h in range(2):
                    head = 2 * pu + h
                    o_ps = st["oA"] if h == 0 else st["oB"]
                    nc.tensor.matmul(
                        o_ps[0:HD + 1, :],
                        lhsT=v_sb[:, kcb, (HD + 1) * head:(HD + 1) * (head + 1)],
                        rhs=pt[:, h * QW:(h + 1) * QW],
                        start=(kcb == 0),
                        stop=(kcb == KC - 1),
                    )
                if kcb == KC - 1:
                    norm_a(u)
                    deferred.setdefault(g + 3, []).append(
                        lambda u=u: norm_b(u)
                    )
        for ops in deferred.values():
            for op in ops:
                op()

        # ---- output projection ----
        for ec in range(6):
            ps = spool.tile([P, NTOK], F32, tag="spool")
            for p in range(NPAIR):
                for nt in range(2):
                    sl = slice(nt * 512, (nt + 1) * 512)
                    nc.tensor.matmul(
                        ps[:, sl],
                        lhsT=pw_sb[:, p, ec * P:(ec + 1) * P],
                        rhs=op_sb[:, p, sl],
                        start=(p == 0),
                        stop=(p == NPAIR - 1),
                    )
            y_sb = youtp.tile([P, NTOK], F32)
            nc.vector.tensor_scalar_add(y_sb[:], ps[:], pb_sb[:, ec:ec + 1])
            nc.sync.dma_start(out=out_d[ec * P:(ec + 1) * P, :], in_=y_sb[:])

    nc.compile()
    _BUILD_CACHE["nc"] = nc
    return nc


def _prep_inputs(x, qkv_w, q_bias, v_bias, rel_bias_table, proj_w, proj_b,
                 rel_pos_idx):
    x = np.asarray(x, np.float32)
    qkv_w = np.asarray(qkv_w, np.float32)
    q_bias = np.asarray(q_bias, np.float32)
    v_bias = np.asarray(v_bias, np.float32)
    rel_bias_table = np.asarray(rel_bias_table, np.float32)
    proj_w = np.asarray(proj_w, np.float32)
    proj_b = np.asarray(proj_b, np.float32)
    rel_pos_idx = np.asarray(rel_pos_idx, np.int64)

    scale = HD ** -0.5
    wq = qkv_w[:C] * scale
    wqkvT = np.ascontiguousarray(
        np.concatenate([wq, qkv_w[C:]], axis=0).T
    ).astype(BF16NP)

    qk_bias = np.concatenate([q_bias * scale, np.zeros(C, np.float32)])
    qkb = np.ascontiguousarray(qk_bias.reshape(12, P).T)

    vb = v_bias.astype(BF16NP).reshape(1, C)

    # E^T[h, m, n] = exp(bias[h, n, m]); bias[h, n, m] = table[idx[n, m], h]
    A = np.exp(rel_bias_table)[rel_pos_idx]            # (n, m, h)
    ETpre = A.transpose(2, 1, 0)                       # (h, m, n)
    ET = np.ascontiguousarray(
        ETpre.reshape(NPAIR, 2, KC, P, NQT, QW).transpose(0, 4, 2, 3, 1, 5)
    ).astype(BF16NP)

    pwT = np.ascontiguousarray(proj_w.T).astype(BF16NP)
    pbT = np.ascontiguousarray(proj_b.reshape(6, P).T)

    shared = {
        "wqkvT": wqkvT, "qkb": qkb, "vb": vb, "ET": ET,
        "pwT": pwT, "pbT": pbT,
    }
    in_maps = []
    xb16 = x.reshape(NCORES, NTOK, C).astype(BF16NP)
    for b in range(NCORES):
        m = dict(shared)
        m["xT"] = np.ascontiguousarray(xb16[b].T)
        in_maps.append(m)
    return in_maps


def _run(inputs, trace=False):
    import time as _time

    _install_axon_hooks()
    from concourse.bass_utils import run_bass_kernel_spmd

    t0 = _time.time()
    nc = _build()
    print(f"[kernel] build+compile: {_time.time() - t0:.1f}s", flush=True)
    t0 = _time.time()
    in_maps = _prep_inputs(**inputs)
    print(f"[kernel] host prep: {_time.time() - t0:.1f}s", flush=True)
    t0 = _time.time()
    res = run_bass_kernel_spmd(
        nc, in_maps, core_ids=list(range(NCORES)), trace=trace
    )
    print(f"[kernel] hw run: {_time.time() - t0:.1f}s", flush=True)
    outs = [np.asarray(res.results[b]["out"]) for b in range(NCORES)]
    y = np.stack([o.T.reshape(32, 32, C) for o in outs]).astype(np.float32)
    return y, res


def kernel(**inputs) -> np.ndarray:
    y, _ = _run(inputs, trace=False)
    return y


# revision 10
# speedup vs baseline: 1.3580x; 1.1355x over previous
"""BEiT window attention (B=8, N=1024, C=768, 12 heads) on 8 TRN2 NeuronCores.

Sharding: pure data-parallel over batch - one batch element per core, no
collectives.  v2: single software-pipelined emission.

Per-core dataflow (bf16 matmuls, f32 PSUM accumulation): qT/kT computed
feature-major so attention scores are produced directly as S^T (keys on
partitions) and softmax needs no on-device transposes.  P = exp(S^T) * E^T
with E = exp(rel-pos bias) precomputed host-side.  Softmax denominators come
from an all-ones column appended to each V block (row 64 of the attn@v PSUM
accumulators).

v2 structure (vs v1's 3 serial phases):
 - Only qk chunks 0/6 and V chunks 0-5 are computed up front; V6/V7 and the
   ten remaining qkv-projection chunks are interleaved into the attention
   units as PE filler so the tensor engine never idles (and never drops out
   of its high p-state) while the Scalar engine paces the exp chain.
 - Scalar engine runs ONLY the 96 exp ACTs (the pacer at ~1.33us each).
 - exp(S)*E multiplies split between DVE and GpSimd (all-SBUF, so Pool can
   take them); PSUM evacuation (qk copies, o copies) stays on DVE.
 - attn@v is skewed 3 beats behind the scores matmuls so its semaphores are
   always satisfied when the PE reaches it.
 - PSUM budget exactly 8 banks: scores 2x2, qkv-filler 1x2, o-accum 2x1.
"""

import sys
import types

import numpy as np
import ml_dtypes

BF16NP = ml_dtypes.bfloat16

P = 128        # partitions
NTOK = 1024    # tokens per batch element
C = 768        # embed dim
NH = 12        # heads
HD = 64        # head dim
NPAIR = 6      # head pairs
NQT = 2        # query tiles of 512
QW = 512       # query tile width
KC = 8         # key chunks of 128
NCORES = 8
SKEW = 3       # beats between scores and attn@v consumption
NBEAT = NPAIR * NQT * KC  # 96


def _install_axon_hooks():
    """Register the NTFF profile hook module missing from this image's antenv."""
    if "antenv.axon_hooks" in sys.modules:
        return
    try:
        import antenv  # noqa: F401
        from trn_agent_boot.trn_boot import _ntff_profile_via_ctypes

        mod = types.ModuleType("antenv.axon_hooks")
        mod._hook = _ntff_profile_via_ctypes("/opt/axon/libaxon_pjrt.so")
        mod.get_axon_ntff_profile_hook = lambda: mod._hook
        mod.set_axon_ntff_profile_hook = lambda h: setattr(mod, "_hook", h)
        sys.modules["antenv.axon_hooks"] = mod
    except Exception:
        pass


_BUILD_CACHE = {}


def _build():
    if "nc" in _BUILD_CACHE:
        return _BUILD_CACHE["nc"]

    from contextlib import ExitStack

    import concourse.bass as bass
    import concourse.bacc as bacc
    import concourse.mybir as mybir
    import concourse.tile as tile

    BF = mybir.dt.bfloat16
    F32 = mybir.dt.float32
    AF = mybir.ActivationFunctionType

    nc = bacc.Bacc("TRN2", target_bir_lowering=False, debug=False)

    xT_d = nc.dram_tensor("xT", [C, NTOK], BF, kind="ExternalInput").ap()
    wqkvT_d = nc.dram_tensor("wqkvT", [C, 3 * C], BF, kind="ExternalInput").ap()
    qkb_d = nc.dram_tensor("qkb", [P, 12], F32, kind="ExternalInput").ap()
    vb_d = nc.dram_tensor("vb", [1, C], BF, kind="ExternalInput").ap()
    # (pair, qtile, kchunk, key-part, head-in-pair, q)
    ET_d = nc.dram_tensor(
        "ET", [NPAIR, NQT, KC, P, 2, QW], BF, kind="ExternalInput"
    ).ap()
    pwT_d = nc.dram_tensor("pwT", [C, C], BF, kind="ExternalInput").ap()
    pbT_d = nc.dram_tensor("pbT", [P, 6], F32, kind="ExternalInput").ap()
    out_d = nc.dram_tensor("out", [C, NTOK], F32, kind="ExternalOutput").ap()

    units = [(p, qt) for p in range(NPAIR) for qt in range(NQT)]

    with ExitStack() as ctx:
        tc = ctx.enter_context(tile.TileContext(nc))
        const = ctx.enter_context(tc.tile_pool(name="const", bufs=1))
        spool = ctx.enter_context(tc.tile_pool(name="spool", bufs=2, space="PSUM"))
        qkpool = ctx.enter_context(tc.tile_pool(name="qkps", bufs=1, space="PSUM"))
        opool = ctx.enter_context(tc.tile_pool(name="opool", bufs=2, space="PSUM"))
        epool = ctx.enter_context(tc.tile_pool(name="epool", bufs=4))
        prawp = ctx.enter_context(tc.tile_pool(name="praw", bufs=4))
        pfinp = ctx.enter_context(tc.tile_pool(name="pfin", bufs=4))
        ocpp = ctx.enter_context(tc.tile_pool(name="ocp", bufs=2))
        smallp = ctx.enter_context(tc.tile_pool(name="small", bufs=4))
        sinvp = ctx.enter_context(tc.tile_pool(name="sinv", bufs=2))
        stgp = ctx.enter_context(tc.tile_pool(name="stg", bufs=2))
        dramp = ctx.enter_context(tc.tile_pool(name="dram", bufs=4, space="DRAM"))
        youtp = ctx.enter_context(tc.tile_pool(name="yout", bufs=2))

        # ---- persistent SBUF tensors ----
        xT_sb = const.tile([P, 6, NTOK], BF)          # x^T, feature-major
        w_sb = const.tile([P, 6, 3 * C], BF)          # qkv_w^T (q cols pre-scaled)
        qk_sb = const.tile([P, 12, NTOK], BF)         # q^T (chunks 0-5), k^T (6-11)
        v_sb = const.tile([P, KC, NH * (HD + 1)], BF)  # 12x[v_h|1] blocks per kchunk
        op_sb = const.tile([P, NPAIR, NTOK], BF)      # normalized O^T, pair-stacked
        pw_sb = const.tile([P, 6, C], BF)             # proj_w^T
        qkb_sb = const.tile([P, 12], F32)
        pb_sb = const.tile([P, 6], F32)
        vb_sb = const.tile([P, C], BF)

        # ---- input DMAs, priority order (single in-order SP queue) ----
        # half-granularity so the first qk matmuls can start before the full
        # weight set lands
        x_r = xT_d.rearrange("(c p) n -> p c n", p=P)
        w_r = wqkvT_d.rearrange("(c p) m -> p c m", p=P)
        nc.sync.dma_start(out=w_sb[:, 0:3, 0:C], in_=w_r[:, 0:3, 0:C])
        nc.sync.dma_start(out=xT_sb[:, 0:3, :], in_=x_r[:, 0:3, :])
        nc.sync.dma_start(out=qkb_sb[:], in_=qkb_d[:])
        nc.sync.dma_start(out=w_sb[:, 3:6, 0:C], in_=w_r[:, 3:6, 0:C])
        nc.sync.dma_start(out=xT_sb[:, 3:6, :], in_=x_r[:, 3:6, :])
        nc.sync.dma_start(out=w_sb[:, 0:3, C:2 * C], in_=w_r[:, 0:3, C:2 * C])
        nc.sync.dma_start(out=w_sb[:, 3:6, C:2 * C], in_=w_r[:, 3:6, C:2 * C])
        nc.sync.dma_start(out=vb_sb[:], in_=vb_d.broadcast_to((P, C)))
        nc.sync.dma_start(out=w_sb[:, 0:3, 2 * C:3 * C], in_=w_r[:, 0:3, 2 * C:3 * C])
        nc.sync.dma_start(out=w_sb[:, 3:6, 2 * C:3 * C], in_=w_r[:, 3:6, 2 * C:3 * C])

        # ones columns of the V blocks (softmax denominator trick)
        nc.gpsimd.memset(
            v_sb[:].rearrange("p k (h w) -> p k h w", w=HD + 1)[:, :, :, HD:HD + 1],
            1.0,
        )

        # ---- building blocks ----
        def qk_matmul_ops(j, pool, cell, tag):
            ops = []

            def get_tile():
                if "ps" not in cell:
                    cell["ps"] = pool.tile(
                        [P, NTOK], F32, tag=tag, name=tag
                    )
                return cell["ps"]

            for c in range(6):
                for half in range(2):
                    def op(c=c, half=half):
                        ps = get_tile()
                        sl = slice(half * QW, (half + 1) * QW)
                        nc.tensor.matmul(
                            ps[:, sl],
                            lhsT=w_sb[:, c, j * P:(j + 1) * P],
                            rhs=xT_sb[:, c, sl],
                            start=(c == 0),
                            stop=(c == 5),
                        )
                    ops.append(op)
            return ops

        def qk_copy_scalar(j, cell):
            # upfront only: Scalar is idle before the exp chain starts
            nc.scalar.activation(
                out=qk_sb[:, j, :], in_=cell["ps"][:], func=AF.Identity,
                bias=qkb_sb[:, j:j + 1],
            )

        def qk_copy_vector(j, cell):
            nc.vector.tensor_scalar_add(
                qk_sb[:, j, :], cell["ps"][:], qkb_sb[:, j:j + 1]
            )

        def v_matmul_ops(t, pool, cell, tag):
            ops = []

            def get_tile():
                if "ps" not in cell:
                    cell["ps"] = pool.tile(
                        [P, NTOK], F32, tag=tag, name=tag
                    )
                return cell["ps"]

            for c in range(6):
                for off, width in ((0, 512), (512, 256)):
                    def op(c=c, off=off, width=width):
                        ps = get_tile()
                        nc.tensor.matmul(
                            ps[:, off:off + width],
                            lhsT=xT_sb[:, c, t * P:(t + 1) * P],
                            rhs=w_sb[:, c, 2 * C + off:2 * C + off + width],
                            start=(c == 0),
                            stop=(c == 5),
                        )
                    ops.append(op)
            return ops

        def v_add(t, cell):
            nc.vector.tensor_add(
                v_sb[:, t, :].rearrange("p (h w) -> p h w", w=HD + 1)[:, :, 0:HD],
                cell["ps"][:, 0:C].rearrange("p (h w) -> p h w", w=HD),
                vb_sb[:].rearrange("p (h w) -> p h w", w=HD),
            )

        # ---- upfront: qk(0), qk(6), V(0..5) ----
        for j in (0, 6):
            cell = {}
            for op in qk_matmul_ops(j, spool, cell, "spool"):
                op()
            qk_copy_scalar(j, cell)
        for t in range(6):
            cell = {}
            for op in v_matmul_ops(t, spool, cell, "spool"):
                op()
            v_add(t, cell)

        # ---- fill plan: PE work interleaved into the attention beats ----
        def make_qk_fill(j):
            cell = {}
            ops = qk_matmul_ops(j, qkpool, cell, "fillps")
            ops.append(lambda: qk_copy_vector(j, cell))
            return ops

        def make_v_fill(t):
            cell = {}
            ops = v_matmul_ops(t, qkpool, cell, "fillps")
            ops.append(lambda: v_add(t, cell))
            return ops

        fill_plan = {
            0: [("v", 6), ("v", 7)],
            1: [("qk", 7), ("qk", 1)],
            2: [("qk", 8)], 3: [("qk", 2)],
            4: [("qk", 9)], 5: [("qk", 3)],
            6: [("qk", 10)], 7: [("qk", 4)],
            8: [("qk", 11)], 9: [("qk", 5)],
            10: [], 11: [],
        }
        fills_by_beat = {}  # global beat -> [callables]
        for u, chunks in fill_plan.items():
            ops = []
            for kind, idx in chunks:
                ops.extend(make_v_fill(idx) if kind == "v" else make_qk_fill(idx))
            if not ops:
                continue
            per_beat = -(-len(ops) // 8)
            for i, op in enumerate(ops):
                g = u * 8 + min(i // per_beat, 7)
                fills_by_beat.setdefault(g, []).append(op)

        # ---- flat attention loop: 96 beats + SKEW trailing ----
        e_tiles = {}
        ptiles = {}
        unit_state = {}
        deferred = {}  # global beat -> [callables]

        def issue_e(g):
            p, qt = units[g // 8]
            kc = g % 8
            e_sb = epool.tile([P, NTOK], BF, tag="e")
            nc.sync.dma_start(
                out=e_sb[:],
                in_=ET_d[p:p + 1, qt:qt + 1, kc:kc + 1, :, :, :].rearrange(
                    "a b c p h q -> (a p) (b c h q)"
                ),
            )
            e_tiles[g] = e_sb

        def norm_a(u):
            """Evacuate o accumulators + start the reciprocal/broadcast chain."""
            st = unit_state[u]
            ocp = ocpp.tile([P, NTOK], F32, tag="ocp")
            nc.vector.tensor_copy(ocp[0:HD + 1, 0:QW], st["oA"][0:HD + 1, :])
            nc.vector.tensor_copy(ocp[0:HD + 1, QW:NTOK], st["oB"][0:HD + 1, :])
            # spread the 1024 sums over 8 partitions so the iterative divide
            # runs 8 lanes wide
            rsp = smallp.tile([32, 32], F32, tag="rsp")
            nc.gpsimd.dma_start(out=rsp[:], in_=ocp[HD:HD + 1, :])
            rinv = smallp.tile([32, 32], F32, tag="rinv")
            nc.vector.reciprocal(rinv[:], rsp[:])
            dscr = dramp.tile([1, NTOK], F32, tag="ds")
            nc.gpsimd.dma_start(out=dscr[:], in_=rinv[:])
            sinv = sinvp.tile([HD, NTOK], F32, tag="sinv")
            nc.gpsimd.dma_start(
                out=sinv[:], in_=dscr[:].broadcast_to((HD, NTOK))
            )
            st["ocp"] = ocp
            st["sinv"] = sinv

        def norm_b(u):
            """Normalize into op_sb (deferred so the DVE never blocks on the
            reciprocal/broadcast latency chain)."""
            p, qt = units[u]
            st = unit_state[u]
            ocp, sinv = st["ocp"], st["sinv"]
            nc.vector.tensor_mul(
                op_sb[0:HD, p, qt * QW:(qt + 1) * QW],
                ocp[0:HD, 0:QW],
                sinv[:, 0:QW],
            )
            stage = stgp.tile([HD, QW], BF, tag="stage")
            nc.vector.tensor_mul(stage[:], ocp[0:HD, QW:NTOK], sinv[:, QW:NTOK])
            nc.gpsimd.dma_start(
                out=op_sb[HD:P, p, qt * QW:(qt + 1) * QW], in_=stage[:]
            )

        issue_e(0)
        issue_e(1)
        for g in range(NBEAT + SKEW):
            for op in deferred.pop(g, ()):
                op()
            if g < NBEAT:
                if g + 2 < NBEAT:
                    issue_e(g + 2)
                p, qt = units[g // 8]
                kc = g % 8
                # scores: S^T tile [128 keys, 2 heads x 512 queries]
                s_ps = spool.tile([P, NTOK], F32, tag="spool")
                for h in range(2):
                    hh = HD * h
                    nc.tensor.matmul(
                        s_ps[:, h * QW:(h + 1) * QW],
                        lhsT=qk_sb[hh:hh + HD, 6 + p, kc * P:(kc + 1) * P],
                        rhs=qk_sb[hh:hh + HD, p, qt * QW:(qt + 1) * QW],
                        start=True,
                        stop=True,
                    )
                praw = prawp.tile([P, NTOK], BF, tag="praw")
                nc.scalar.activation(out=praw[:], in_=s_ps[:], func=AF.Exp)
                ptile = pfinp.tile([P, NTOK], BF, tag="pfin")
                nc.vector.tensor_mul(ptile[:], praw[:], e_tiles.pop(g)[:])
                ptiles[g] = ptile
                for op in fills_by_beat.pop(g, ()):
                    op()
            if g == 72:
                # proj weights, needed from ~t=150us; issue mid-stream
                nc.sync.dma_start(
                    out=pw_sb[:], in_=pwT_d.rearrange("(c p) m -> p c m", p=P)
                )
                nc.sync.dma_start(out=pb_sb[:], in_=pbT_d[:])
            b = g - SKEW
            if b >= 0:
                u = b // 8
                kcb = b % 8
                if kcb == 0:
                    oA = opool.tile([P, QW], F32, tag="o", name="oA")
                    oB = opool.tile([P, QW], F32, tag="o", name="oB")
                    unit_state[u] = {"oA": oA, "oB": oB}
                st = unit_state[u]
                pt = ptiles.pop(b)
                pu, _ = units[u]
                for h in range(2):
                    head = 2 * pu + h
                    o_ps = st["oA"] if h == 0 else st["oB"]
                    nc.tensor.matmul(
                        o_ps[0:HD + 1, :],
                        lhsT=v_sb[:, kcb, (HD + 1) * head:(HD + 1) * (head + 1)],
                        rhs=pt[:, h * QW:(h + 1) * QW],
                        start=(kcb == 0),
                        stop=(kcb == KC - 1),
                    )
                if kcb == KC - 1:
                    norm_a(u)
                    deferred.setdefault(g + 3, []).append(
                        lambda u=u: norm_b(u)
                    )
        for ops in deferred.values():
            for op in ops:
                op()

        # ---- output projection ----
        # two groups of 3 psum accumulators; pairs 0-4 accumulate while the
        # last unit's normalization chain drains, pair 5 appended once its
        # op_sb lands, so the PE never idles behind the norm latency
        def proj_group(ecs):
            tiles = {}
            for i, ec in enumerate(ecs):
                pool = qkpool if i == 2 else spool
                tiles[ec] = pool.tile(
                    [P, NTOK], F32, tag="fillps" if i == 2 else "spool",
                    name="projps",
                )
            for plast in (False, True):
                for ec in ecs:
                    for p in ((5,) if plast else range(5)):
                        for nt in range(2):
                            sl = slice(nt * 512, (nt + 1) * 512)
                            nc.tensor.matmul(
                                tiles[ec][:, sl],
                                lhsT=pw_sb[:, p, ec * P:(ec + 1) * P],
                                rhs=op_sb[:, p, sl],
                                start=(p == 0),
                                stop=(p == NPAIR - 1),
                            )
            for ec in ecs:
                y_sb = youtp.tile([P, NTOK], F32, tag="y", name="y_sb")
                nc.vector.tensor_scalar_add(
                    y_sb[:], tiles[ec][:], pb_sb[:, ec:ec + 1]
                )
                nc.sync.dma_start(
                    out=out_d[ec * P:(ec + 1) * P, :], in_=y_sb[:]
                )

        proj_group([0, 1, 2])
        proj_group([3, 4, 5])

    nc.compile()
    _BUILD_CACHE["nc"] = nc
    return nc


def _prep_inputs(x, qkv_w, q_bias, v_bias, rel_bias_table, proj_w, proj_b,
                 rel_pos_idx):
    x = np.asarray(x, np.float32)
    qkv_w = np.asarray(qkv_w, np.float32)
    q_bias = np.asarray(q_bias, np.float32)
    v_bias = np.asarray(v_bias, np.float32)
    rel_bias_table = np.asarray(rel_bias_table, np.float32)
    proj_w = np.asarray(proj_w, np.float32)
    proj_b = np.asarray(proj_b, np.float32)
    rel_pos_idx = np.asarray(rel_pos_idx, np.int64)

    scale = HD ** -0.5
    wq = qkv_w[:C] * scale
    wqkvT = np.ascontiguousarray(
        np.concatenate([wq, qkv_w[C:]], axis=0).T
    ).astype(BF16NP)

    qk_bias = np.concatenate([q_bias * scale, np.zeros(C, np.float32)])
    qkb = np.ascontiguousarray(qk_bias.reshape(12, P).T)

    vb = v_bias.astype(BF16NP).reshape(1, C)

    # E^T[h, m, n] = exp(bias[h, n, m]); bias[h, n, m] = table[idx[n, m], h]
    A = np.exp(rel_bias_table)[rel_pos_idx]            # (n, m, h)
    ETpre = A.transpose(2, 1, 0)                       # (h, m, n)
    ET = np.ascontiguousarray(
        ETpre.reshape(NPAIR, 2, KC, P, NQT, QW).transpose(0, 4, 2, 3, 1, 5)
    ).astype(BF16NP)

    pwT = np.ascontiguousarray(proj_w.T).astype(BF16NP)
    pbT = np.ascontiguousarray(proj_b.reshape(6, P).T)

    shared = {
        "wqkvT": wqkvT, "qkb": qkb, "vb": vb, "ET": ET,
        "pwT": pwT, "pbT": pbT,
    }
    in_maps = []
    xb16 = x.reshape(NCORES, NTOK, C).astype(BF16NP)
    for b in range(NCORES):
        m = dict(shared)
        m["xT"] = np.ascontiguousarray(xb16[b].T)
        in_maps.append(m)
    return in_maps


def _run(inputs, trace=False):
    import time as _time

    _install_axon_hooks()
    from concourse.bass_utils import run_bass_kernel_spmd

    t0 = _time.time()
    nc = _build()
    print(f"[kernel] build+compile: {_time.time() - t0:.1f}s", flush=True)
    t0 = _time.time()
    in_maps = _prep_inputs(**inputs)
    print(f"[kernel] host prep: {_time.time() - t0:.1f}s", flush=True)
    t0 = _time.time()
    res = run_bass_kernel_spmd(
        nc, in_maps, core_ids=list(range(NCORES)), trace=trace
    )
    print(f"[kernel] hw run: {_time.time() - t0:.1f}s", flush=True)
    outs = [np.asarray(res.results[b]["out"]) for b in range(NCORES)]
    y = np.stack([o.T.reshape(32, 32, C) for o in outs]).astype(np.float32)
    return y, res


def kernel(**inputs) -> np.ndarray:
    y, _ = _run(inputs, trace=False)
    return y


# revision 12
# speedup vs baseline: 1.4304x; 1.0533x over previous
"""BEiT window attention (B=8, N=1024, C=768, 12 heads) on 8 TRN2 NeuronCores.

Sharding: pure data-parallel over batch - one batch element per core, no
collectives.  v2: single software-pipelined emission.

Per-core dataflow (bf16 matmuls, f32 PSUM accumulation): qT/kT computed
feature-major so attention scores are produced directly as S^T (keys on
partitions) and softmax needs no on-device transposes.  P = exp(S^T) * E^T
with E = exp(rel-pos bias) precomputed host-side.  Softmax denominators come
from an all-ones column appended to each V block (row 64 of the attn@v PSUM
accumulators).

v2 structure (vs v1's 3 serial phases):
 - Only qk chunks 0/6 and V chunks 0-5 are computed up front; V6/V7 and the
   ten remaining qkv-projection chunks are interleaved into the attention
   units as PE filler so the tensor engine never idles (and never drops out
   of its high p-state) while the Scalar engine paces the exp chain.
 - Scalar engine runs ONLY the 96 exp ACTs (the pacer at ~1.33us each).
 - exp(S)*E multiplies split between DVE and GpSimd (all-SBUF, so Pool can
   take them); PSUM evacuation (qk copies, o copies) stays on DVE.
 - attn@v is skewed 3 beats behind the scores matmuls so its semaphores are
   always satisfied when the PE reaches it.
 - PSUM budget exactly 8 banks: scores 2x2, qkv-filler 1x2, o-accum 2x1.
"""

import sys
import types

import numpy as np
import ml_dtypes

BF16NP = ml_dtypes.bfloat16

P = 128        # partitions
NTOK = 1024    # tokens per batch element
C = 768        # embed dim
NH = 12        # heads
HD = 64        # head dim
NPAIR = 6      # head pairs
NQT = 2        # query tiles of 512
QW = 512       # query tile width
KC = 8         # key chunks of 128
NCORES = 8
SKEW = 3       # beats between scores and attn@v consumption
NBEAT = NPAIR * NQT * KC  # 96


def _install_axon_hooks():
    """Register the NTFF profile hook module missing from this image's antenv."""
    if "antenv.axon_hooks" in sys.modules:
        return
    try:
        import antenv  # noqa: F401
        from trn_agent_boot.trn_boot import _ntff_profile_via_ctypes

        mod = types.ModuleType("antenv.axon_hooks")
        mod._hook = _ntff_profile_via_ctypes("/opt/axon/libaxon_pjrt.so")
        mod.get_axon_ntff_profile_hook = lambda: mod._hook
        mod.set_axon_ntff_profile_hook = lambda h: setattr(mod, "_hook", h)
        sys.modules["antenv.axon_hooks"] = mod
    except Exception:
        pass


_BUILD_CACHE = {}


def _build():
    if "nc" in _BUILD_CACHE:
        return _BUILD_CACHE["nc"]

    from contextlib import ExitStack

    import concourse.bass as bass
    import concourse.bacc as bacc
    import concourse.mybir as mybir
    import concourse.tile as tile

    BF = mybir.dt.bfloat16
    F32 = mybir.dt.float32
    AF = mybir.ActivationFunctionType

    nc = bacc.Bacc("TRN2", target_bir_lowering=False, debug=False)

    xT_d = nc.dram_tensor("xT", [C, NTOK], BF, kind="ExternalInput").ap()
    wqkvT_d = nc.dram_tensor("wqkvT", [C, 3 * C], BF, kind="ExternalInput").ap()
    qkb_d = nc.dram_tensor("qkb", [P, 12], F32, kind="ExternalInput").ap()
    vb_d = nc.dram_tensor("vb", [1, C], BF, kind="ExternalInput").ap()
    # (pair, qtile, kchunk, key-part, head-in-pair, q)
    ET_d = nc.dram_tensor(
        "ET", [NPAIR, NQT, KC, P, 2, QW], BF, kind="ExternalInput"
    ).ap()
    pwT_d = nc.dram_tensor("pwT", [C, C], BF, kind="ExternalInput").ap()
    pbT_d = nc.dram_tensor("pbT", [P, 6], F32, kind="ExternalInput").ap()
    out_d = nc.dram_tensor("out", [C, NTOK], F32, kind="ExternalOutput").ap()

    units = [(p, qt) for p in range(NPAIR) for qt in range(NQT)]

    with ExitStack() as ctx:
        tc = ctx.enter_context(tile.TileContext(nc))
        const = ctx.enter_context(tc.tile_pool(name="const", bufs=1))
        spool = ctx.enter_context(tc.tile_pool(name="spool", bufs=2, space="PSUM"))
        qkpool = ctx.enter_context(tc.tile_pool(name="qkps", bufs=1, space="PSUM"))
        opool = ctx.enter_context(tc.tile_pool(name="opool", bufs=3, space="PSUM"))
        epool = ctx.enter_context(tc.tile_pool(name="epool", bufs=5))
        prawp = ctx.enter_context(tc.tile_pool(name="praw", bufs=5))
        pfinp = ctx.enter_context(tc.tile_pool(name="pfin", bufs=5))
        ocpp = ctx.enter_context(tc.tile_pool(name="ocp", bufs=2))
        smallp = ctx.enter_context(tc.tile_pool(name="small", bufs=4))
        sinvp = ctx.enter_context(tc.tile_pool(name="sinv", bufs=2))
        stgp = ctx.enter_context(tc.tile_pool(name="stg", bufs=2))
        dramp = ctx.enter_context(tc.tile_pool(name="dram", bufs=4, space="DRAM"))
        youtp = ctx.enter_context(tc.tile_pool(name="yout", bufs=2))

        # ---- persistent SBUF tensors ----
        xT_sb = const.tile([P, 6, NTOK], BF)          # x^T, feature-major
        w_sb = const.tile([P, 6, 3 * C], BF)          # qkv_w^T (q cols pre-scaled)
        qk_sb = const.tile([P, 12, NTOK], BF)         # q^T (chunks 0-5), k^T (6-11)
        v_sb = const.tile([P, KC, NH * (HD + 1)], BF)  # 12x[v_h|1] blocks per kchunk
        op_sb = const.tile([P, NPAIR, NTOK], BF)      # normalized O^T, pair-stacked
        pw_sb = const.tile([P, 6, C], BF)             # proj_w^T
        qkb_sb = const.tile([P, 12], F32)
        pb_sb = const.tile([P, 6], F32)
        vb_sb = const.tile([P, C], BF)

        # ---- input DMAs, priority order (single in-order SP queue) ----
        # half-granularity so the first qk matmuls can start before the full
        # weight set lands
        x_r = xT_d.rearrange("(c p) n -> p c n", p=P)
        w_r = wqkvT_d.rearrange("(c p) m -> p c m", p=P)
        for c in range(6):
            nc.sync.dma_start(out=w_sb[:, c, 0:C], in_=w_r[:, c, 0:C])
            nc.sync.dma_start(out=xT_sb[:, c, :], in_=x_r[:, c, :])
        nc.sync.dma_start(out=qkb_sb[:], in_=qkb_d[:])
        nc.sync.dma_start(out=w_sb[:, 0:3, C:2 * C], in_=w_r[:, 0:3, C:2 * C])
        nc.sync.dma_start(out=w_sb[:, 3:6, C:2 * C], in_=w_r[:, 3:6, C:2 * C])
        nc.sync.dma_start(out=w_sb[:, 0:3, 2 * C:3 * C], in_=w_r[:, 0:3, 2 * C:3 * C])
        nc.sync.dma_start(out=w_sb[:, 3:6, 2 * C:3 * C], in_=w_r[:, 3:6, 2 * C:3 * C])
        nc.sync.dma_start(out=vb_sb[:], in_=vb_d.broadcast_to((P, C)))

        # ones columns of the V blocks (softmax denominator trick)
        nc.gpsimd.memset(
            v_sb[:].rearrange("p k (h w) -> p k h w", w=HD + 1)[:, :, :, HD:HD + 1],
            1.0,
        )

        # ---- building blocks ----
        def qk_matmul_ops(j, pool, cell, tag):
            ops = []

            def get_tile():
                if "ps" not in cell:
                    cell["ps"] = pool.tile(
                        [P, NTOK], F32, tag=tag, name=tag
                    )
                return cell["ps"]

            for c in range(6):
                for half in range(2):
                    def op(c=c, half=half):
                        ps = get_tile()
                        sl = slice(half * QW, (half + 1) * QW)
                        nc.tensor.matmul(
                            ps[:, sl],
                            lhsT=w_sb[:, c, j * P:(j + 1) * P],
                            rhs=xT_sb[:, c, sl],
                            start=(c == 0),
                            stop=(c == 5),
                        )
                    ops.append(op)
            return ops

        def qk_copy_scalar(j, cell):
            # upfront only: Scalar is idle before the exp chain starts
            nc.scalar.activation(
                out=qk_sb[:, j, :], in_=cell["ps"][:], func=AF.Identity,
                bias=qkb_sb[:, j:j + 1],
            )

        def qk_copy_vector(j, cell):
            nc.vector.tensor_scalar_add(
                qk_sb[:, j, :], cell["ps"][:], qkb_sb[:, j:j + 1]
            )

        def v_matmul_ops(t, pool, cell, tag):
            ops = []

            def get_tile():
                if "ps" not in cell:
                    cell["ps"] = pool.tile(
                        [P, NTOK], F32, tag=tag, name=tag
                    )
                return cell["ps"]

            for c in range(6):
                for off, width in ((0, 512), (512, 256)):
                    def op(c=c, off=off, width=width):
                        ps = get_tile()
                        nc.tensor.matmul(
                            ps[:, off:off + width],
                            lhsT=xT_sb[:, c, t * P:(t + 1) * P],
                            rhs=w_sb[:, c, 2 * C + off:2 * C + off + width],
                            start=(c == 0),
                            stop=(c == 5),
                        )
                    ops.append(op)
            return ops

        def v_add(t, cell):
            nc.vector.tensor_add(
                v_sb[:, t, :].rearrange("p (h w) -> p h w", w=HD + 1)[:, :, 0:HD],
                cell["ps"][:, 0:C].rearrange("p (h w) -> p h w", w=HD),
                vb_sb[:].rearrange("p (h w) -> p h w", w=HD),
            )

        # ---- upfront: qk(0), qk(6), V(0..5) ----
        for j in (0, 6):
            cell = {}
            for op in qk_matmul_ops(j, spool, cell, "spool"):
                op()
            qk_copy_scalar(j, cell)
        for t in range(6):
            cell = {}
            for op in v_matmul_ops(t, spool, cell, "spool"):
                op()
            v_add(t, cell)

        # ---- fill plan: PE work interleaved into the attention beats ----
        # each chunk is two half-accumulations through a single 1-bank psum
        # tile, so the filler pool costs 1 bank and the freed bank deepens
        # the o-accumulator pool
        def fill_tile(cell):
            if "ps" not in cell:
                cell["ps"] = qkpool.tile(
                    [P, QW], F32, tag="fillps", name="fillps"
                )
            return cell["ps"]

        def make_qk_fill(j):
            ops = []
            for half in range(2):
                cell = {}
                for c in range(6):
                    def op(c=c, half=half, cell=cell):
                        ps = fill_tile(cell)
                        nc.tensor.matmul(
                            ps[:],
                            lhsT=w_sb[:, c, j * P:(j + 1) * P],
                            rhs=xT_sb[:, c, half * QW:(half + 1) * QW],
                            start=(c == 0),
                            stop=(c == 5),
                        )
                    ops.append(op)

                def copy_op(half=half, cell=cell):
                    nc.vector.tensor_scalar_add(
                        qk_sb[:, j, half * QW:(half + 1) * QW],
                        cell["ps"][:],
                        qkb_sb[:, j:j + 1],
                    )
                ops.append(copy_op)
            return ops

        def make_v_fill(t):
            ops = []
            for half in range(2):
                cell = {}
                width = 512 if half == 0 else 256
                for c in range(6):
                    def op(c=c, half=half, width=width, cell=cell):
                        ps = fill_tile(cell)
                        off = 2 * C + half * QW
                        nc.tensor.matmul(
                            ps[:, 0:width],
                            lhsT=xT_sb[:, c, t * P:(t + 1) * P],
                            rhs=w_sb[:, c, off:off + width],
                            start=(c == 0),
                            stop=(c == 5),
                        )
                    ops.append(op)

                def add_op(half=half, width=width, cell=cell):
                    nh0 = half * 8
                    nh = width // HD
                    nc.vector.tensor_add(
                        v_sb[:, t, :].rearrange(
                            "p (h w) -> p h w", w=HD + 1
                        )[:, nh0:nh0 + nh, 0:HD],
                        cell["ps"][:, 0:width].rearrange(
                            "p (h w) -> p h w", w=HD
                        ),
                        vb_sb[:, half * QW:half * QW + width].rearrange(
                            "p (h w) -> p h w", w=HD
                        ),
                    )
                ops.append(add_op)
            return ops

        fill_plan = {
            0: [("v", 6), ("v", 7)],
            1: [("qk", 7), ("qk", 1)],
            2: [("qk", 8)], 3: [("qk", 2)],
            4: [("qk", 9)], 5: [("qk", 3)],
            6: [("qk", 10)], 7: [("qk", 4)],
            8: [("qk", 11)], 9: [("qk", 5)],
            10: [], 11: [],
        }
        fills_by_beat = {}  # global beat -> [callables]
        for u, chunks in fill_plan.items():
            ops = []
            for kind, idx in chunks:
                ops.extend(make_v_fill(idx) if kind == "v" else make_qk_fill(idx))
            if not ops:
                continue
            per_beat = -(-len(ops) // 8)
            for i, op in enumerate(ops):
                g = u * 8 + min(i // per_beat, 7)
                fills_by_beat.setdefault(g, []).append(op)

        # ---- flat attention loop: 96 beats + SKEW trailing ----
        e_tiles = {}
        ptiles = {}
        unit_state = {}
        deferred = {}  # global beat -> [callables]

        def issue_e(g):
            p, qt = units[g // 8]
            kc = g % 8
            e_sb = epool.tile([P, NTOK], BF, tag="e")
            nc.sync.dma_start(
                out=e_sb[:],
                in_=ET_d[p:p + 1, qt:qt + 1, kc:kc + 1, :, :, :].rearrange(
                    "a b c p h q -> (a p) (b c h q)"
                ),
            )
            e_tiles[g] = e_sb

        def norm_a(u):
            """Evacuate o accumulators + start the reciprocal/broadcast chain."""
            st = unit_state[u]
            ocp = ocpp.tile([P, NTOK], F32, tag="ocp")
            nc.vector.tensor_copy(ocp[0:HD + 1, 0:QW], st["oA"][0:HD + 1, :])
            nc.vector.tensor_copy(ocp[0:HD + 1, QW:NTOK], st["oB"][0:HD + 1, :])
            # spread the 1024 sums over 8 partitions so the iterative divide
            # runs 8 lanes wide
            rsp = smallp.tile([32, 32], F32, tag="rsp")
            nc.gpsimd.dma_start(out=rsp[:], in_=ocp[HD:HD + 1, :])
            rinv = smallp.tile([32, 32], F32, tag="rinv")
            nc.vector.reciprocal(rinv[:], rsp[:])
            dscr = dramp.tile([1, NTOK], F32, tag="ds")
            nc.gpsimd.dma_start(out=dscr[:], in_=rinv[:])
            sinv = sinvp.tile([HD, NTOK], F32, tag="sinv")
            nc.gpsimd.dma_start(
                out=sinv[:], in_=dscr[:].broadcast_to((HD, NTOK))
            )
            st["ocp"] = ocp
            st["sinv"] = sinv

        def norm_b(u):
            """Normalize into op_sb (deferred so the DVE never blocks on the
            reciprocal/broadcast latency chain)."""
            p, qt = units[u]
            st = unit_state[u]
            ocp, sinv = st["ocp"], st["sinv"]
            nc.vector.tensor_mul(
                op_sb[0:HD, p, qt * QW:(qt + 1) * QW],
                ocp[0:HD, 0:QW],
                sinv[:, 0:QW],
            )
            stage = stgp.tile([HD, QW], BF, tag="stage")
            nc.vector.tensor_mul(stage[:], ocp[0:HD, QW:NTOK], sinv[:, QW:NTOK])
            nc.gpsimd.dma_start(
                out=op_sb[HD:P, p, qt * QW:(qt + 1) * QW], in_=stage[:]
            )

        def av_beat(b):
            u = b // 8
            kcb = b % 8
            if kcb == 0:
                oA = opool.tile([P, QW], F32, tag="o", name="oA")
                oB = opool.tile([P, QW], F32, tag="o", name="oB")
                unit_state[u] = {"oA": oA, "oB": oB}
            st = unit_state[u]
            pt = ptiles.pop(b)
            pu, _ = units[u]
            for h in range(2):
                head = 2 * pu + h
                o_ps = st["oA"] if h == 0 else st["oB"]
                nc.tensor.matmul(
                    o_ps[0:HD + 1, :],
                    lhsT=v_sb[:, kcb, (HD + 1) * head:(HD + 1) * (head + 1)],
                    rhs=pt[:, h * QW:(h + 1) * QW],
                    start=(kcb == 0),
                    stop=(kcb == KC - 1),
                )
            return u if kcb == KC - 1 else None

        issue_e(0)
        issue_e(1)
        for g in range(NBEAT + SKEW):
            for op in deferred.pop(g, ()):
                op()
            b = g - SKEW
            if b >= 0:
                udone = av_beat(b)
                if udone is not None:
                    norm_a(udone)
                    deferred.setdefault(g + 5, []).append(
                        lambda u=udone: norm_b(u)
                    )
            if g < NBEAT:
                if g + 2 < NBEAT:
                    issue_e(g + 2)
                p, qt = units[g // 8]
                kc = g % 8
                # scores: S^T tile [128 keys, 2 heads x 512 queries]
                s_ps = spool.tile([P, NTOK], F32, tag="spool")
                for h in range(2):
                    hh = HD * h
                    nc.tensor.matmul(
                        s_ps[:, h * QW:(h + 1) * QW],
                        lhsT=qk_sb[hh:hh + HD, 6 + p, kc * P:(kc + 1) * P],
                        rhs=qk_sb[hh:hh + HD, p, qt * QW:(qt + 1) * QW],
                        start=True,
                        stop=True,
                    )
                praw = prawp.tile([P, NTOK], BF, tag="praw")
                nc.scalar.activation(out=praw[:], in_=s_ps[:], func=AF.Exp)
                ptile = pfinp.tile([P, NTOK], BF, tag="pfin")
                nc.vector.tensor_mul(ptile[:], praw[:], e_tiles.pop(g)[:])
                ptiles[g] = ptile
                for op in fills_by_beat.pop(g, ()):
                    op()
            if g == 72:
                # proj weights, needed from ~t=150us; issue mid-stream
                nc.sync.dma_start(
                    out=pw_sb[:], in_=pwT_d.rearrange("(c p) m -> p c m", p=P)
                )
                nc.sync.dma_start(out=pb_sb[:], in_=pbT_d[:])
        for ops in deferred.values():
            for op in ops:
                op()

        # ---- output projection ----
        # two groups of 3 psum accumulators; pairs 0-4 accumulate while the
        # last unit's normalization chain drains, pair 5 appended once its
        # op_sb lands, so the PE never idles behind the norm latency
        def proj_group(ecs):
            ta = spool.tile([P, NTOK], F32, tag="spool", name="projps")
            tb = spool.tile([P, NTOK], F32, tag="spool", name="projps")
            th0 = qkpool.tile([P, QW], F32, tag="fillps", name="projh0")
            th1 = opool.tile([P, QW], F32, tag="o", name="projh1")
            tiles = {
                ecs[0]: (ta[:, 0:QW], ta[:, QW:NTOK]),
                ecs[1]: (tb[:, 0:QW], tb[:, QW:NTOK]),
                ecs[2]: (th0[:], th1[:]),
            }
            for plast in (False, True):
                for ec in ecs:
                    for p in ((5,) if plast else range(5)):
                        for nt in range(2):
                            nc.tensor.matmul(
                                tiles[ec][nt],
                                lhsT=pw_sb[:, p, ec * P:(ec + 1) * P],
                                rhs=op_sb[:, p, nt * QW:(nt + 1) * QW],
                                start=(p == 0),
                                stop=(p == NPAIR - 1),
                            )
            for ec in ecs:
                y_sb = youtp.tile([P, NTOK], F32, tag="y", name="y_sb")
                for nt in range(2):
                    nc.vector.tensor_scalar_add(
                        y_sb[:, nt * QW:(nt + 1) * QW],
                        tiles[ec][nt],
                        pb_sb[:, ec:ec + 1],
                    )
                nc.sync.dma_start(
                    out=out_d[ec * P:(ec + 1) * P, :], in_=y_sb[:]
                )

        proj_group([0, 1, 2])
        proj_group([3, 4, 5])

    nc.compile()
    _BUILD_CACHE["nc"] = nc
    return nc


def _prep_inputs(x, qkv_w, q_bias, v_bias, rel_bias_table, proj_w, proj_b,
                 rel_pos_idx):
    x = np.asarray(x, np.float32)
    qkv_w = np.asarray(qkv_w, np.float32)
    q_bias = np.asarray(q_bias, np.float32)
    v_bias = np.asarray(v_bias, np.float32)
    rel_bias_table = np.asarray(rel_bias_table, np.float32)
    proj_w = np.asarray(proj_w, np.float32)
    proj_b = np.asarray(proj_b, np.float32)
    rel_pos_idx = np.asarray(rel_pos_idx, np.int64)

    scale = HD ** -0.5
    wq = qkv_w[:C] * scale
    wqkvT = np.ascontiguousarray(
        np.concatenate([wq, qkv_w[C:]], axis=0).T
    ).astype(BF16NP)

    qk_bias = np.concatenate([q_bias * scale, np.zeros(C, np.float32)])
    qkb = np.ascontiguousarray(qk_bias.reshape(12, P).T)

    vb = v_bias.astype(BF16NP).reshape(1, C)

    # E^T[h, m, n] = exp(bias[h, n, m]); bias[h, n, m] = table[idx[n, m], h]
    A = np.exp(rel_bias_table)[rel_pos_idx]            # (n, m, h)
    ETpre = A.transpose(2, 1, 0)                       # (h, m, n)
    ET = np.ascontiguousarray(
        ETpre.reshape(NPAIR, 2, KC, P, NQT, QW).transpose(0, 4, 2, 3, 1, 5)
    ).astype(BF16NP)

    pwT = np.ascontiguousarray(proj_w.T).astype(BF16NP)
    pbT = np.ascontiguousarray(proj_b.reshape(6, P).T)

    shared = {
        "wqkvT": wqkvT, "qkb": qkb, "vb": vb, "ET": ET,
        "pwT": pwT, "pbT": pbT,
    }
    in_maps = []
    xb16 = x.reshape(NCORES, NTOK, C).astype(BF16NP)
    for b in range(NCORES):
        m = dict(shared)
        m["xT"] = np.ascontiguousarray(xb16[b].T)
        in_maps.append(m)
    return in_maps


def _run(inputs, trace=False):
    import time as _time

    _install_axon_hooks()
    from concourse.bass_utils import run_bass_kernel_spmd

    t0 = _time.time()
    nc = _build()
    print(f"[kernel] build+compile: {_time.time() - t0:.1f}s", flush=True)
    t0 = _time.time()
    in_maps = _prep_inputs(**inputs)
    print(f"[kernel] host prep: {_time.time() - t0:.1f}s", flush=True)
    t0 = _time.time()
    res = run_bass_kernel_spmd(
        nc, in_maps, core_ids=list(range(NCORES)), trace=trace
    )
    print(f"[kernel] hw run: {_time.time() - t0:.1f}s", flush=True)
    outs = [np.asarray(res.results[b]["out"]) for b in range(NCORES)]
    y = np.stack([o.T.reshape(32, 32, C) for o in outs]).astype(np.float32)
    return y, res


def kernel(**inputs) -> np.ndarray:
    y, _ = _run(inputs, trace=False)
    return y
